# revision 11
# baseline (speedup 1.0000x reference)
"""Multi-head causal attention (B=4, N=2048, D=1024, H=16) on 8 NeuronCores.

Sharding: data-parallel over batch (4) x tensor-parallel over heads (2 halves
of 8 heads each), Megatron-style.  Core c handles batch c//2 and head-half
c%2: Q/K/V projections restricted to its 512 output dims, full causal
attention for its 8 heads, and a partial output projection; the host sums the
two partials per batch.

v3 design:
  - x transposed + bf16-converted on the HOST (xqT/xkT/xvT [1024, 2048]):
    no PE transposes on device; all device matmuls bf16 (full PE rate, FWL
    weight loads, half DMA).
  - chunk-interleaved: project Q/K/V for 512-row chunk sc, attention for
    q-chunk sc (causal => K/V chunks <= sc), out-projection, repeat.
    Separate PSUM pools for projections (2 banks), out-proj (2), scores (2),
    AV accumulators (2) so chunk sc+1 projections overlap attention sc.
  - scores PSUM tile [128, 2, 512] holds BOTH head-halves of one k-tile
    (bank0=h0, bank1=h1): one exp instruction covers both heads.
  - diagonal 512-block computed triangularly (widths 512/384/256/128) with
    [128,128] identity-matmul mask adds; extras packed into 2 score tiles.
  - softmax row sums via ones-column in Vp (AV matmul M=65); normalization
    via reciprocal + partition-broadcast + multiply into aoT.
  - DMAs spread: x/weight loads on SP + ACT queues, output stores via
    gpsimd SWDGE.
"""

import numpy as np
import ml_dtypes

import concourse.bass as bass
import concourse.bacc as bacc
import concourse.mybir as mybir
import concourse.tile as tile
from concourse.bass_utils import run_bass_kernel_spmd

B, N, D, H = 4, 2048, 1024, 16
HD = 64          # head dim
DH = 512         # per-core slice of D (8 heads)
NT = N // 128    # 16 seq tiles
CT = D // 128    # 8 feature tiles
NC = 4           # 512-row chunks
NEG = -1e30

f32 = mybir.dt.float32
bf16 = mybir.dt.bfloat16
EXP = mybir.ActivationFunctionType.Exp


def _load_xt(nc, h, pools, sc, name):
    xt = pools["xt"].tile([128, CT, 512], bf16, tag=f"xt{name}", name="xt")
    nc.sync.dma_start(
        xt[:],
        h["x" + name][:].rearrange("(o p) f -> p o f", p=128)
        [:, :, sc * 512:(sc + 1) * 512],
    )
    return xt


def _proj_chunk(nc, h, pools, sc, name, xt):
    """Project one input chunk: Q/K -> [dh, n] transposed; V -> Vp blocks."""
    w_t = h["w_" + name]
    ps_pool = pools["mmp"]
    if name in ("q", "k"):
        if name == "q":
            dst = pools["qt"].tile([128, 4, 512], bf16, tag="qt", name="qt")
            h["QT"] = dst
        else:
            dst = pools["kt"].tile([128, 4, 512], bf16, tag=f"kt{sc}",
                                   name="kt")
            h["KT"][sc] = dst
        for dt_ in range(4):
            ps = ps_pool.tile([128, 512], f32, tag="mmp", name="psp")
            for ct in range(CT):
                nc.tensor.matmul(
                    ps[:],
                    lhsT=w_t[:, ct, dt_ * 128:(dt_ + 1) * 128],
                    rhs=xt[:, ct, :],
                    start=(ct == 0), stop=(ct == CT - 1),
                )
            nc.vector.tensor_copy(dst[:, dt_, :], ps[:])
    else:  # v
        dst = pools["vp"].tile([128, 4, 8, 65], bf16, tag=f"vp{sc}",
                               name="vp")
        h["Vp"][sc] = dst
        for st in range(4):
            ps = ps_pool.tile([128, 512], f32, tag="mmp", name="psp")
            for ct in range(CT):
                nc.tensor.matmul(
                    ps[:],
                    lhsT=xt[:, ct, st * 128:(st + 1) * 128],
                    rhs=w_t[:, ct, :],
                    start=(ct == 0), stop=(ct == CT - 1),
                )
            src = ps[:].rearrange("p (h d) -> p h d", h=8)
            nc.vector.tensor_copy(dst[:, st, :, 0:64], src)
        nc.vector.memset(dst[:, :, :, 64:65], 1.0)


def _attn_chunk(nc, h, pools, sc, phase):
    """Causal attention for q-chunk sc over k chunks 0..sc, into aoT."""
    QT, KT, Vp = h["QT"], h["KT"], h["Vp"]
    mask_t, ident_t = h["mask_t"], h["ident_t"]
    aoT = pools["ao"].tile([128, 4, 512], bf16, tag="ao", name="ao")
    h["aoT"] = aoT

    for pr in range(4):
        av = None
        if phase != "noav":
            av = [
                pools["av"].tile([65, 512], f32, tag="av", name="av0"),
                pools["av"].tile([65, 512], f32, tag="av", name="av1"),
            ]
        # full-width k-tiles 0 .. sc*4 (the last one, jj0, needs a mask on
        # its first 128 cols); one s tile per k-tile, bank per head-half.
        for kt in range(sc * 4 + 1):
            cc, lk = divmod(kt, 4)
            diag = kt == sc * 4
            s_ps = pools["s"].tile([128, 2, 512], f32, tag="s", name="s")
            for h2 in (0, 1):
                p0, p1 = h2 * 64, h2 * 64 + 64
                nc.tensor.matmul(
                    s_ps[:, h2, :],
                    lhsT=KT[cc][p0:p1, pr, lk * 128:(lk + 1) * 128],
                    rhs=QT[p0:p1, pr, :],
                    start=True, stop=not diag,
                )
                if diag:
                    nc.tensor.matmul(
                        s_ps[:, h2, 0:128],
                        lhsT=ident_t[:],
                        rhs=mask_t[:],
                        start=False, stop=True,
                    )
            p_sb = pools["p"].tile([128, 2, 512], bf16, tag="p", name="p")
            if phase == "noexp":
                nc.scalar.activation(
                    p_sb[:, :, 0:1], s_ps[:, :, 0:1], EXP, scale=0.125)
            else:
                nc.scalar.activation(p_sb[:], s_ps[:], EXP, scale=0.125)
            if phase != "noav":
                for h2 in (0, 1):
                    nc.tensor.matmul(
                        av[h2][:],
                        lhsT=Vp[cc][:, lk, pr * 2 + h2, :],
                        rhs=p_sb[:, h2, :],
                        start=(kt == 0), stop=False,
                    )

        # diagonal extras jj1..jj3 (valid q-cols 384/256/128), two s tiles:
        #  ex1: bank h2 = [jj1 cols 0:384 | jj3 cols 384:512]
        #  ex2: bank 0  = [h0 jj2 cols 0:256 | h1 jj2 cols 256:512]
        ex1 = pools["s"].tile([128, 2, 512], f32, tag="s", name="ex1")
        for h2 in (0, 1):
            p0, p1 = h2 * 64, h2 * 64 + 64
            for c0, w, jj in ((0, 384, 1), (384, 128, 3)):
                q0 = jj * 128
                nc.tensor.matmul(
                    ex1[:, h2, c0:c0 + w],
                    lhsT=KT[sc][p0:p1, pr, jj * 128:(jj + 1) * 128],
                    rhs=QT[p0:p1, pr, q0:512],
                    start=True, stop=False,
                )
                nc.tensor.matmul(
                    ex1[:, h2, c0:c0 + 128],
                    lhsT=ident_t[:], rhs=mask_t[:],
                    start=False, stop=True,
                )
        ex2 = pools["s"].tile([128, 2, 512], f32, tag="s", name="ex2")
        for h2 in (0, 1):
            p0, p1 = h2 * 64, h2 * 64 + 64
            c0 = h2 * 256
            nc.tensor.matmul(
                ex2[:, 0, c0:c0 + 256],
                lhsT=KT[sc][p0:p1, pr, 256:384],
                rhs=QT[p0:p1, pr, 256:512],
                start=True, stop=False,
            )
            nc.tensor.matmul(
                ex2[:, 0, c0:c0 + 128],
                lhsT=ident_t[:], rhs=mask_t[:],
                start=False, stop=True,
            )
        px1 = pools["p"].tile([128, 2, 512], bf16, tag="p", name="px1")
        px2 = pools["p"].tile([128, 2, 512], bf16, tag="p", name="px2")
        if phase == "noexp":
            nc.scalar.activation(px1[:, :, 0:1], ex1[:, :, 0:1], EXP, scale=0.125)
            nc.scalar.activation(px2[:, 0, 0:1], ex2[:, 0, 0:1], EXP, scale=0.125)
        else:
            nc.scalar.activation(px1[:], ex1[:], EXP, scale=0.125)
            nc.scalar.activation(px2[:, 0, :], ex2[:, 0, :], EXP, scale=0.125)
        for h2 in (0, 1) if phase != "noav" else ():
            hh = pr * 2 + h2
            nc.tensor.matmul(
                av[h2][:, 128:512],
                lhsT=Vp[sc][:, 1, hh, :], rhs=px1[:, h2, 0:384],
                start=False, stop=False,
            )
            nc.tensor.matmul(
                av[h2][:, 256:512],
                lhsT=Vp[sc][:, 2, hh, :],
                rhs=px2[:, 0, h2 * 256:h2 * 256 + 256],
                start=False, stop=False,
            )
            nc.tensor.matmul(
                av[h2][:, 384:512],
                lhsT=Vp[sc][:, 3, hh, :], rhs=px1[:, h2, 384:512],
                start=False, stop=True,
            )

        # normalize: aoT[head rows, :] = av[0:64] / av[64]
        if phase == "noav":
            nc.vector.memset(aoT[:, pr, :], 0.01)
            continue
        for h2 in (0, 1):
            r_t = pools["small"].tile([1, 512], bf16, tag="r", name="r_t")
            with nc.allow_low_precision(reason="softmax denom recip in bf16"):
                nc.vector.reciprocal(r_t[:], av[h2][64:65, :])
            R_ps = pools["s"].tile([128, 2, 512], f32, tag="s", name="R_ps")
            nc.tensor.matmul(
                R_ps[0:64, 0, :], lhsT=h["ones_t"][:], rhs=r_t[:],
                start=True, stop=True,
            )
            av_sb = pools["small"].tile([64, 512], f32, tag="avsb",
                                        name="av_sb")
            nc.vector.tensor_copy(av_sb[:], av[h2][0:64, :])
            nc.vector.tensor_mul(
                aoT[h2 * 64:h2 * 64 + 64, pr, :], av_sb[:],
                R_ps[0:64, 0, :])


def _outproj_chunk(nc, h, pools, sc):
    aoT, woT_t, out = h["aoT"], h["woT_t"], h["out"]
    for ntl in range(4):
        o_sb = pools["o"].tile([128, D], f32, tag="o", name="o_sb")
        ps_o = pools["s"].tile([128, 2, 512], f32, tag="s", name="ps_o")
        for dc in range(2):
            for jt in range(4):
                nc.tensor.matmul(
                    ps_o[:, dc, :],
                    lhsT=aoT[:, jt, ntl * 128:(ntl + 1) * 128],
                    rhs=woT_t[:, jt, dc * 512:(dc + 1) * 512],
                    start=(jt == 0), stop=(jt == 3),
                )
        nc.vector.tensor_copy(
            o_sb[:].rearrange("p (a b) -> p a b", a=2), ps_o[:])
        nt = sc * 4 + ntl
        nc.sync.dma_start(out[nt * 128:(nt + 1) * 128, :], o_sb[:])


def _build_nc(reps=1, phase="all"):
    nc = bacc.Bacc(None, target_bir_lowering=False)
    h = {}
    for nm in ("xq", "xk", "xv"):
        h[nm] = nc.declare_dram_parameter(nm + "T", [D, N], bf16,
                                          isOutput=False)
    for nm in ("q", "k", "v"):
        h["wd_" + nm] = nc.declare_dram_parameter(
            "w" + nm, [D, DH], bf16, isOutput=False)
    h["woT"] = nc.declare_dram_parameter("woT", [DH, D], bf16, isOutput=False)
    h["mask"] = nc.declare_dram_parameter("mask128", [128, 128], bf16,
                                          isOutput=False)
    h["ident"] = nc.declare_dram_parameter("ident", [128, 128], bf16,
                                           isOutput=False)
    h["out"] = nc.declare_dram_parameter("out", [N, D], f32, isOutput=True)

    with tile.TileContext(nc) as tc:
        with (
            tc.tile_pool(name="consts", bufs=1) as consts,
            tc.tile_pool(name="kt", bufs=1) as kt_pool,
            tc.tile_pool(name="vp", bufs=1) as vp_pool,
            tc.tile_pool(name="qt", bufs=2) as qt_pool,
            tc.tile_pool(name="xt", bufs=2) as xt_pool,
            tc.tile_pool(name="ao", bufs=2) as ao_pool,
            tc.tile_pool(name="p", bufs=4) as p_pool,
            tc.tile_pool(name="small", bufs=4) as small_pool,
            tc.tile_pool(name="o", bufs=3) as o_pool,
            tc.tile_pool(name="ps_mmp", bufs=2, space="PSUM") as mmp_pool,
            tc.tile_pool(name="ps_s", bufs=2, space="PSUM") as s_pool,
            tc.tile_pool(name="ps_av", bufs=2, space="PSUM") as av_pool,
        ):
            # small consts via the ACT DMA queue (SP queue carries x inputs)
            h["mask_t"] = consts.tile([128, 128], bf16, name="mask_t")
            nc.scalar.dma_start(h["mask_t"][:], h["mask"][:])
            h["ident_t"] = consts.tile([128, 128], bf16, name="ident_t")
            nc.scalar.dma_start(h["ident_t"][:], h["ident"][:])
            h["ones_t"] = consts.tile([1, 64], bf16, name="ones_t")
            nc.vector.memset(h["ones_t"][:], 1.0)
            for nm in ("q", "k", "v"):
                w_t = consts.tile([128, CT, DH], bf16, name=f"w_{nm}")
                eng = nc.sync if nm == "q" else nc.scalar
                eng.dma_start(
                    w_t[:],
                    h["wd_" + nm][:].rearrange("(o p) f -> p o f", p=128),
                )
                h["w_" + nm] = w_t
            h["woT_t"] = consts.tile([128, 4, D], bf16, name="woT_t")
            nc.scalar.dma_start(
                h["woT_t"][:],
                h["woT"][:].rearrange("(o p) f -> p o f", p=128),
            )

            pools = {
                "kt": kt_pool, "vp": vp_pool, "qt": qt_pool, "xt": xt_pool,
                "ao": ao_pool, "p": p_pool, "small": small_pool, "o": o_pool,
                "mmp": mmp_pool, "s": s_pool, "av": av_pool,
            }

            for rep in range(reps):
                h["KT"] = [None] * NC
                h["Vp"] = [None] * NC
                for sc in range(NC):
                    if phase == "attn":
                        qt = qt_pool.tile([128, 4, 512], bf16, tag="qt",
                                          name="qt")
                        nc.vector.memset(qt[:], 0.01)
                        h["QT"] = qt
                        kt = kt_pool.tile([128, 4, 512], bf16, tag=f"kt{sc}",
                                          name="kt")
                        nc.vector.memset(kt[:], 0.01)
                        h["KT"][sc] = kt
                        vp = vp_pool.tile([128, 4, 8, 65], bf16,
                                          tag=f"vp{sc}", name="vp")
                        nc.vector.memset(vp[:], 0.01)
                        h["Vp"][sc] = vp
                    else:
                        xts = {nm: _load_xt(nc, h, pools, sc, nm)
                               for nm in ("q", "k", "v")}
                        for nm in ("q", "k", "v"):
                            _proj_chunk(nc, h, pools, sc, nm, xts[nm])
                    if phase == "proj":
                        continue
                    _attn_chunk(nc, h, pools, sc, phase)
                    _outproj_chunk(nc, h, pools, sc)
                if phase == "proj":
                    o_sb = o_pool.tile([128, D], f32, tag="o", name="o_sb")
                    nc.vector.tensor_copy(
                        o_sb[:, 0:256], h["QT"][:, 0, :].bitcast(f32)[:, 0:256])
                    nc.sync.dma_start(h["out"][0:128, :], o_sb[:])
    nc.compile()
    return nc


_NC = None


def _get_nc():
    global _NC
    if _NC is None:
        _NC = _build_nc()
    return _NC


def _make_in_maps(q, k, v, Wq, Wk, Wv, Wo):
    q = np.asarray(q, np.float32)
    k = np.asarray(k, np.float32)
    v = np.asarray(v, np.float32)
    Wq = np.asarray(Wq, np.float32)
    Wk = np.asarray(Wk, np.float32)
    Wv = np.asarray(Wv, np.float32)
    Wo = np.asarray(Wo, np.float32)
    bf = ml_dtypes.bfloat16

    pp = np.arange(128)[:, None]
    jj = np.arange(128)[None, :]
    mask128 = np.where(pp > jj, NEG, 0.0).astype(bf)
    ident = np.eye(128, dtype=bf)

    xT = {}
    for b in range(B):
        xT[("q", b)] = np.ascontiguousarray(q[b].T).astype(bf)
        xT[("k", b)] = np.ascontiguousarray(k[b].T).astype(bf)
        xT[("v", b)] = np.ascontiguousarray(v[b].T).astype(bf)

    in_maps = []
    for c in range(8):
        b, hh = divmod(c, 2)
        sl = slice(hh * DH, (hh + 1) * DH)
        in_maps.append({
            "xqT": xT[("q", b)],
            "xkT": xT[("k", b)],
            "xvT": xT[("v", b)],
            "wq": np.ascontiguousarray(Wq[sl, :].T).astype(bf),
            "wk": np.ascontiguousarray(Wk[sl, :].T).astype(bf),
            "wv": np.ascontiguousarray(Wv[sl, :].T).astype(bf),
            "woT": np.ascontiguousarray(Wo[:, sl].T).astype(bf),
            "mask128": mask128,
            "ident": ident,
        })
    return in_maps


def kernel(q, k, v, Wq, Wk, Wv, Wo):
    nc = _get_nc()
    in_maps = _make_in_maps(q, k, v, Wq, Wk, Wv, Wo)
    res = run_bass_kernel_spmd(nc, in_maps, core_ids=list(range(8)))
    out = np.empty((B, N, D), np.float32)
    for b in range(B):
        out[b] = res.results[2 * b]["out"] + res.results[2 * b + 1]["out"]
    return out


# revision 13
# speedup vs baseline: 1.3696x; 1.3696x over previous
"""Multi-head causal attention (B=4, N=2048, D=1024, H=16) on 8 NeuronCores.

Sharding: data-parallel over batch (4) x tensor-parallel over heads (2 halves
of 8 heads each), Megatron-style.  Core c handles batch c//2 and head-half
c%2: Q/K/V projections restricted to its 512 output dims, full causal
attention for its 8 heads, and a partial output projection; the host sums the
two partials per batch.

v3 design:
  - x transposed + bf16-converted on the HOST (xqT/xkT/xvT [1024, 2048]):
    no PE transposes on device; all device matmuls bf16 (full PE rate, FWL
    weight loads, half DMA).
  - chunk-interleaved: project Q/K/V for 512-row chunk sc, attention for
    q-chunk sc (causal => K/V chunks <= sc), out-projection, repeat.
    Separate PSUM pools for projections (2 banks), out-proj (2), scores (2),
    AV accumulators (2) so chunk sc+1 projections overlap attention sc.
  - scores PSUM tile [128, 2, 512] holds BOTH head-halves of one k-tile
    (bank0=h0, bank1=h1): one exp instruction covers both heads.
  - diagonal 512-block computed triangularly (widths 512/384/256/128) with
    [128,128] identity-matmul mask adds; extras packed into 2 score tiles.
  - softmax row sums via ones-column in Vp (AV matmul M=65); normalization
    via reciprocal + partition-broadcast + multiply into aoT.
  - DMAs spread: x/weight loads on SP + ACT queues, output stores via
    gpsimd SWDGE.
"""

import numpy as np
import ml_dtypes

import concourse.bass as bass
import concourse.bacc as bacc
import concourse.mybir as mybir
import concourse.tile as tile
from concourse.bass_utils import run_bass_kernel_spmd

B, N, D, H = 4, 2048, 1024, 16
HD = 64          # head dim
DH = 512         # per-core slice of D (8 heads)
NT = N // 128    # 16 seq tiles
CT = D // 128    # 8 feature tiles
NC = 4           # 512-row chunks
NEG = -1e30

f32 = mybir.dt.float32
bf16 = mybir.dt.bfloat16
EXP = mybir.ActivationFunctionType.Exp


def _load_xt(nc, h, pools, sc, name):
    xt = pools["xt"].tile([128, CT, 512], bf16, tag=f"xt{name}", name="xt")
    nc.sync.dma_start(
        xt[:],
        h["x" + name][:].rearrange("(o p) f -> p o f", p=128)
        [:, :, sc * 512:(sc + 1) * 512],
    )
    return xt


def _proj_chunk(nc, h, pools, sc, name, xt):
    """Project one input chunk: Q/K -> [dh, n] transposed; V -> Vp blocks.

    PSUM comes from the shared rotating "s" pool; each [128, 2, 512] tile
    holds two 128-dh output blocks (16 matmuls) to maximize work per slot.
    """
    w_t = h["w_" + name]
    if name in ("q", "k"):
        if name == "q":
            dst = pools["qt"].tile([128, 4, 512], bf16, tag="qt", name="qt")
            h["QT"] = dst
        else:
            dst = pools["kt"].tile([128, 4, 512], bf16, tag=f"kt{sc}",
                                   name="kt")
            h["KT"][sc] = dst
        for dt0 in (0, 2):
            ps = pools["s"].tile([128, 2, 512], f32, tag="s", name="psp")
            for i in range(2):
                for ct in range(CT):
                    nc.tensor.matmul(
                        ps[:, i, :],
                        lhsT=w_t[:, ct, (dt0 + i) * 128:(dt0 + i + 1) * 128],
                        rhs=xt[:, ct, :],
                        start=(ct == 0), stop=(ct == CT - 1),
                    )
            nc.vector.tensor_copy(dst[:, dt0:dt0 + 2, :], ps[:])
    else:  # v
        dst = pools["vp"].tile([128, 4, 8, 65], bf16, tag=f"vp{sc}",
                               name="vp")
        h["Vp"][sc] = dst
        for st0 in (0, 2):
            ps = pools["s"].tile([128, 2, 512], f32, tag="s", name="psp")
            for i in range(2):
                for ct in range(CT):
                    nc.tensor.matmul(
                        ps[:, i, :],
                        lhsT=xt[:, ct, (st0 + i) * 128:(st0 + i + 1) * 128],
                        rhs=w_t[:, ct, :],
                        start=(ct == 0), stop=(ct == CT - 1),
                    )
            src = ps[:].rearrange("p a (h d) -> p a h d", h=8)
            nc.vector.tensor_copy(dst[:, st0:st0 + 2, :, 0:64], src)
        nc.vector.memset(dst[:, :, :, 64:65], 1.0)


def _attn_chunk(nc, h, pools, sc, phase):
    """Causal attention for q-chunk sc over k chunks 0..sc, into aoT."""
    QT, KT, Vp = h["QT"], h["KT"], h["Vp"]
    mask_t, ident_t = h["mask_t"], h["ident_t"]
    aoT = pools["ao"].tile([128, 4, 512], bf16, tag="ao", name="ao")
    h["aoT"] = aoT

    for pr in range(4):
        av = None
        if phase != "noav":
            av = [
                pools["av"].tile([65, 512], f32, tag="av", name="av0"),
                pools["av"].tile([65, 512], f32, tag="av", name="av1"),
            ]
        # full-width k-tiles 0 .. sc*4 (the last one, jj0, needs a mask on
        # its first 128 cols); one s tile per k-tile, bank per head-half.
        for kt in range(sc * 4 + 1):
            cc, lk = divmod(kt, 4)
            diag = kt == sc * 4
            s_ps = pools["s"].tile([128, 2, 512], f32, tag="s", name="s")
            for h2 in (0, 1):
                p0, p1 = h2 * 64, h2 * 64 + 64
                nc.tensor.matmul(
                    s_ps[:, h2, :],
                    lhsT=KT[cc][p0:p1, pr, lk * 128:(lk + 1) * 128],
                    rhs=QT[p0:p1, pr, :],
                    start=True, stop=not diag,
                )
                if diag:
                    nc.tensor.matmul(
                        s_ps[:, h2, 0:128],
                        lhsT=ident_t[:],
                        rhs=mask_t[:],
                        start=False, stop=True,
                    )
            p_sb = pools["p"].tile([128, 2, 512], bf16, tag="p", name="p")
            if phase == "noexp":
                nc.scalar.activation(
                    p_sb[:, :, 0:1], s_ps[:, :, 0:1], EXP, scale=0.125)
            else:
                nc.scalar.activation(p_sb[:], s_ps[:], EXP, scale=0.125)
            if phase != "noav":
                for h2 in (0, 1):
                    nc.tensor.matmul(
                        av[h2][:],
                        lhsT=Vp[cc][:, lk, pr * 2 + h2, :],
                        rhs=p_sb[:, h2, :],
                        start=(kt == 0), stop=False,
                    )

        # diagonal extras jj1..jj3 (valid q-cols 384/256/128), two s tiles:
        #  ex1: bank h2 = [jj1 cols 0:384 | jj3 cols 384:512]
        #  ex2: bank 0  = [h0 jj2 cols 0:256 | h1 jj2 cols 256:512]
        ex1 = pools["s"].tile([128, 2, 512], f32, tag="s", name="ex1")
        for h2 in (0, 1):
            p0, p1 = h2 * 64, h2 * 64 + 64
            for c0, w, jj in ((0, 384, 1), (384, 128, 3)):
                q0 = jj * 128
                nc.tensor.matmul(
                    ex1[:, h2, c0:c0 + w],
                    lhsT=KT[sc][p0:p1, pr, jj * 128:(jj + 1) * 128],
                    rhs=QT[p0:p1, pr, q0:512],
                    start=True, stop=False,
                )
                nc.tensor.matmul(
                    ex1[:, h2, c0:c0 + 128],
                    lhsT=ident_t[:], rhs=mask_t[:],
                    start=False, stop=True,
                )
        ex2 = pools["s"].tile([128, 2, 512], f32, tag="s", name="ex2")
        for h2 in (0, 1):
            p0, p1 = h2 * 64, h2 * 64 + 64
            c0 = h2 * 256
            nc.tensor.matmul(
                ex2[:, 0, c0:c0 + 256],
                lhsT=KT[sc][p0:p1, pr, 256:384],
                rhs=QT[p0:p1, pr, 256:512],
                start=True, stop=False,
            )
            nc.tensor.matmul(
                ex2[:, 0, c0:c0 + 128],
                lhsT=ident_t[:], rhs=mask_t[:],
                start=False, stop=True,
            )
        px1 = pools["p"].tile([128, 2, 512], bf16, tag="p", name="px1")
        px2 = pools["p"].tile([128, 2, 512], bf16, tag="p", name="px2")
        if phase == "noexp":
            nc.scalar.activation(px1[:, :, 0:1], ex1[:, :, 0:1], EXP, scale=0.125)
            nc.scalar.activation(px2[:, 0, 0:1], ex2[:, 0, 0:1], EXP, scale=0.125)
        else:
            nc.scalar.activation(px1[:], ex1[:], EXP, scale=0.125)
            nc.scalar.activation(px2[:, 0, :], ex2[:, 0, :], EXP, scale=0.125)
        for h2 in (0, 1) if phase != "noav" else ():
            hh = pr * 2 + h2
            nc.tensor.matmul(
                av[h2][:, 128:512],
                lhsT=Vp[sc][:, 1, hh, :], rhs=px1[:, h2, 0:384],
                start=False, stop=False,
            )
            nc.tensor.matmul(
                av[h2][:, 256:512],
                lhsT=Vp[sc][:, 2, hh, :],
                rhs=px2[:, 0, h2 * 256:h2 * 256 + 256],
                start=False, stop=False,
            )
            nc.tensor.matmul(
                av[h2][:, 384:512],
                lhsT=Vp[sc][:, 3, hh, :], rhs=px1[:, h2, 384:512],
                start=False, stop=True,
            )

        # normalize: aoT[head rows, :] = av[0:64] / av[64]
        if phase == "noav":
            nc.vector.memset(aoT[:, pr, :], 0.01)
            continue
        for h2 in (0, 1):
            av_sb = pools["small"].tile([65, 512], f32, tag="avsb",
                                        name="av_sb")
            nc.vector.tensor_copy(av_sb[:], av[h2][:])
            r_t = pools["small"].tile([1, 512], f32, tag="r", name="r_t")
            nc.vector.reciprocal(r_t[:], av_sb[64:65, :])
            R_t = pools["small"].tile([64, 512], f32, tag="R", name="R_t")
            nc.gpsimd.partition_broadcast(R_t[:], r_t[:])
            nc.vector.tensor_mul(
                aoT[h2 * 64:h2 * 64 + 64, pr, :], av_sb[0:64, :], R_t[:])


def _outproj_chunk(nc, h, pools, sc):
    aoT, woT_t, out = h["aoT"], h["woT_t"], h["out"]
    for ntl in range(4):
        o_sb = pools["o"].tile([128, D], f32, tag="o", name="o_sb")
        ps_o = pools["s"].tile([128, 2, 512], f32, tag="s", name="ps_o")
        for dc in range(2):
            for jt in range(4):
                nc.tensor.matmul(
                    ps_o[:, dc, :],
                    lhsT=aoT[:, jt, ntl * 128:(ntl + 1) * 128],
                    rhs=woT_t[:, jt, dc * 512:(dc + 1) * 512],
                    start=(jt == 0), stop=(jt == 3),
                )
        nc.vector.tensor_copy(
            o_sb[:].rearrange("p (a b) -> p a b", a=2), ps_o[:])
        nt = sc * 4 + ntl
        nc.sync.dma_start(out[nt * 128:(nt + 1) * 128, :], o_sb[:])


def _build_nc(reps=1, phase="all"):
    nc = bacc.Bacc(None, target_bir_lowering=False)
    h = {}
    for nm in ("xq", "xk", "xv"):
        h[nm] = nc.declare_dram_parameter(nm + "T", [D, N], bf16,
                                          isOutput=False)
    for nm in ("q", "k", "v"):
        h["wd_" + nm] = nc.declare_dram_parameter(
            "w" + nm, [D, DH], bf16, isOutput=False)
    h["woT"] = nc.declare_dram_parameter("woT", [DH, D], bf16, isOutput=False)
    h["mask"] = nc.declare_dram_parameter("mask128", [128, 128], bf16,
                                          isOutput=False)
    h["ident"] = nc.declare_dram_parameter("ident", [128, 128], bf16,
                                           isOutput=False)
    h["out"] = nc.declare_dram_parameter("out", [N, D], f32, isOutput=True)

    with tile.TileContext(nc) as tc:
        with (
            tc.tile_pool(name="consts", bufs=1) as consts,
            tc.tile_pool(name="kt", bufs=1) as kt_pool,
            tc.tile_pool(name="vp", bufs=1) as vp_pool,
            tc.tile_pool(name="qt", bufs=2) as qt_pool,
            tc.tile_pool(name="xt", bufs=2) as xt_pool,
            tc.tile_pool(name="ao", bufs=2) as ao_pool,
            tc.tile_pool(name="p", bufs=6) as p_pool,
            tc.tile_pool(name="small", bufs=4) as small_pool,
            tc.tile_pool(name="o", bufs=3) as o_pool,
            tc.tile_pool(name="ps_s", bufs=3, space="PSUM") as s_pool,
            tc.tile_pool(name="ps_av", bufs=2, space="PSUM") as av_pool,
        ):
            # small consts via the ACT DMA queue (SP queue carries x inputs)
            h["mask_t"] = consts.tile([128, 128], bf16, name="mask_t")
            nc.scalar.dma_start(h["mask_t"][:], h["mask"][:])
            h["ident_t"] = consts.tile([128, 128], bf16, name="ident_t")
            nc.scalar.dma_start(h["ident_t"][:], h["ident"][:])
            h["ones_t"] = consts.tile([1, 64], bf16, name="ones_t")
            nc.vector.memset(h["ones_t"][:], 1.0)
            for nm in ("q", "k", "v"):
                w_t = consts.tile([128, CT, DH], bf16, name=f"w_{nm}")
                eng = nc.sync if nm == "q" else nc.scalar
                eng.dma_start(
                    w_t[:],
                    h["wd_" + nm][:].rearrange("(o p) f -> p o f", p=128),
                )
                h["w_" + nm] = w_t
            h["woT_t"] = consts.tile([128, 4, D], bf16, name="woT_t")
            nc.scalar.dma_start(
                h["woT_t"][:],
                h["woT"][:].rearrange("(o p) f -> p o f", p=128),
            )

            pools = {
                "kt": kt_pool, "vp": vp_pool, "qt": qt_pool, "xt": xt_pool,
                "ao": ao_pool, "p": p_pool, "small": small_pool, "o": o_pool,
                "s": s_pool, "av": av_pool,
            }

            for rep in range(reps):
                h["KT"] = [None] * NC
                h["Vp"] = [None] * NC
                for sc in range(NC):
                    if phase == "attn":
                        qt = qt_pool.tile([128, 4, 512], bf16, tag="qt",
                                          name="qt")
                        nc.vector.memset(qt[:], 0.01)
                        h["QT"] = qt
                        kt = kt_pool.tile([128, 4, 512], bf16, tag=f"kt{sc}",
                                          name="kt")
                        nc.vector.memset(kt[:], 0.01)
                        h["KT"][sc] = kt
                        vp = vp_pool.tile([128, 4, 8, 65], bf16,
                                          tag=f"vp{sc}", name="vp")
                        nc.vector.memset(vp[:], 0.01)
                        h["Vp"][sc] = vp
                    else:
                        xts = {nm: _load_xt(nc, h, pools, sc, nm)
                               for nm in ("q", "k", "v")}
                        for nm in ("q", "k", "v"):
                            _proj_chunk(nc, h, pools, sc, nm, xts[nm])
                    if phase == "proj":
                        continue
                    _attn_chunk(nc, h, pools, sc, phase)
                    _outproj_chunk(nc, h, pools, sc)
                if phase == "proj":
                    o_sb = o_pool.tile([128, D], f32, tag="o", name="o_sb")
                    nc.vector.tensor_copy(
                        o_sb[:, 0:256], h["QT"][:, 0, :].bitcast(f32)[:, 0:256])
                    nc.sync.dma_start(h["out"][0:128, :], o_sb[:])
    nc.compile()
    return nc


_NC = None


def _get_nc():
    global _NC
    if _NC is None:
        _NC = _build_nc()
    return _NC


def _make_in_maps(q, k, v, Wq, Wk, Wv, Wo):
    q = np.asarray(q, np.float32)
    k = np.asarray(k, np.float32)
    v = np.asarray(v, np.float32)
    Wq = np.asarray(Wq, np.float32)
    Wk = np.asarray(Wk, np.float32)
    Wv = np.asarray(Wv, np.float32)
    Wo = np.asarray(Wo, np.float32)
    bf = ml_dtypes.bfloat16

    pp = np.arange(128)[:, None]
    jj = np.arange(128)[None, :]
    mask128 = np.where(pp > jj, NEG, 0.0).astype(bf)
    ident = np.eye(128, dtype=bf)

    xT = {}
    for b in range(B):
        xT[("q", b)] = np.ascontiguousarray(q[b].T).astype(bf)
        xT[("k", b)] = np.ascontiguousarray(k[b].T).astype(bf)
        xT[("v", b)] = np.ascontiguousarray(v[b].T).astype(bf)

    in_maps = []
    for c in range(8):
        b, hh = divmod(c, 2)
        sl = slice(hh * DH, (hh + 1) * DH)
        in_maps.append({
            "xqT": xT[("q", b)],
            "xkT": xT[("k", b)],
            "xvT": xT[("v", b)],
            "wq": np.ascontiguousarray(Wq[sl, :].T).astype(bf),
            "wk": np.ascontiguousarray(Wk[sl, :].T).astype(bf),
            "wv": np.ascontiguousarray(Wv[sl, :].T).astype(bf),
            "woT": np.ascontiguousarray(Wo[:, sl].T).astype(bf),
            "mask128": mask128,
            "ident": ident,
        })
    return in_maps


def kernel(q, k, v, Wq, Wk, Wv, Wo):
    nc = _get_nc()
    in_maps = _make_in_maps(q, k, v, Wq, Wk, Wv, Wo)
    res = run_bass_kernel_spmd(nc, in_maps, core_ids=list(range(8)))
    out = np.empty((B, N, D), np.float32)
    for b in range(B):
        out[b] = res.results[2 * b]["out"] + res.results[2 * b + 1]["out"]
    return out
